# revision 1
# baseline (speedup 1.0000x reference)
"""Trainium2 Bass kernel for ClinicalStateFormationOperator.

Full-input contract: kernel(**inputs) takes the complete (unsharded) numpy
inputs and returns the full [B, T, V, D] output. Internally the work is
sharded across 8 NeuronCores as (batch, head-group): core c handles batch
c//2 and heads (c%2)*4 .. (c%2)*4+3. Each core computes its 4 heads'
attention and the partial output projection; the host sums the two partial
projections per batch and adds the output bias.

Math notes (per core, N = T*V = 1536 tokens, head_dim=64, obs_dim=16).
Scores are computed transposed (keys on partitions, queries free) in ONE
K=128 matmul per [128k x 512q] tile by packing four contraction groups:

  rows  0: 64  kT_h            |  qT_h * scale         (content)
  rows 64: 80  okT_h           |  oqT_h * obs_scale    (observation)
  rows 80:112  [K%32==j]       |  VB_h[Q%32, j]        (variable bias)
  rows112:128  A_hj[s,K]=rtb_h[16j+s-K//32+47] | [(Q//32)%16==s]  (time bias;
               the 16-row basis spans the 16 time bins of q-chunk j, so the
               A rows are re-DMA'd into the k-pack once per (head, q-chunk))

  E^T = exp(scores^T)  (no max-subtraction: |scores| <~ 5, fp32-safe)
  [out^T; denom_rep] = [v | ones_64]^T @ E^T  (ones columns replicate the
        softmax denominator across 64 partitions -> aligned divide)
  OT = out^T * reciprocal(denom_rep)
  y_partial = OT^T_heads @ Wo_rows   (host sums core pairs + bo)

All matmuls run in float32r (full-rate fp32 PE mode, ~1e-4 rel err).
q/k/v/obs biases are zero in this problem's setup_inputs; a with-bias
variant (K=1 bias matmuls into the projection psums) is built only if a
nonzero bias is ever passed.
"""

import numpy as np

import concourse.bass as bass
import concourse.mybir as mybir
import concourse.tile as tile
from concourse.bass_utils import run_bass_kernel_spmd

V = 32
T = 48
D = 512
H = 8
HD = D // H          # 64
OD = 16
B = 4
N = T * V            # 1536
HPC = 4              # heads per core
NCORES = 8
SCALE = 1.0 / np.sqrt(HD)
OBS_SCALE = 1.0 / np.sqrt(OD)

F32 = mybir.dt.float32
F32R = mybir.dt.float32r
EXP = mybir.ActivationFunctionType.Exp

KC = N // 128        # 12 key chunks of 128
QC = N // 512        # 3 query chunks of 512


def _split_waits(nc, max_waits=1):
    """Walrus in this container allows only one sync-wait slot per
    instruction; spill extra waits onto preceding same-engine NoOps."""
    def fix_bb(bb):
        changed = False
        new = []
        for inst in bb.instructions:
            si = inst.sync_info
            if si is not None and len(si.on_wait) > max_waits:
                waits = list(si.on_wait)
                for w in waits[:-max_waits]:
                    new.append(mybir.InstNoOp(
                        name=nc.get_next_instruction_name(),
                        engine=inst.engine, ins=[], outs=[],
                        sync_info=mybir.SyncInfo(on_wait=[w], on_update=[])))
                    changed = True
                si.on_wait = waits[-max_waits:]
            new.append(inst)
        if changed:
            bb.instructions = new
        for sub in getattr(bb, 'blocks', []) or []:
            fix_bb(sub)
    for f in nc.m.functions:
        for bb in f.blocks:
            fix_bb(bb)


def _build(with_bias=False):
    nc = bass.Bass()

    # ---- per-core DRAM I/O (data differs per core, program is SPMD) ----
    fhT = nc.dram_tensor('fhT', [D, N], F32R, kind='ExternalInput')
    foT = nc.dram_tensor('foT', [2, N], F32R, kind='ExternalInput')
    wq = nc.dram_tensor('wq', [D, HPC * HD], F32R, kind='ExternalInput')
    wk = nc.dram_tensor('wk', [D, HPC * HD], F32R, kind='ExternalInput')
    wv = nc.dram_tensor('wv', [D, HPC * HD], F32R, kind='ExternalInput')
    woq = nc.dram_tensor('woq', [2, 128], F32R, kind='ExternalInput')
    wok = nc.dram_tensor('wok', [2, 128], F32R, kind='ExternalInput')
    wo = nc.dram_tensor('wo', [2, 128, D], F32R, kind='ExternalInput')
    # score-bias expansion tables (host-gathered from variable_bias / rtb)
    kstat = nc.dram_tensor('kstat', [32, N], F32R, kind='ExternalInput')
    qstat = nc.dram_tensor('qstat', [HPC, 48, N], F32R, kind='ExternalInput')
    apack = nc.dram_tensor('apack', [HPC, QC, 16, N], F32R,
                           kind='ExternalInput')
    vones = nc.dram_tensor('vones', [128, 64], F32R, kind='ExternalInput')
    if with_bias:
        bqr = nc.dram_tensor('bqr', [1, HPC * HD], F32R, kind='ExternalInput')
        bkr = nc.dram_tensor('bkr', [1, HPC * HD], F32R, kind='ExternalInput')
        bvr = nc.dram_tensor('bvr', [1, HPC * HD], F32R, kind='ExternalInput')
        boqr = nc.dram_tensor('boqr', [1, 128], F32R, kind='ExternalInput')
        bokr = nc.dram_tensor('bokr', [1, 128], F32R, kind='ExternalInput')
        onesd = nc.dram_tensor('onesd', [1, 512], F32R, kind='ExternalInput')
    out = nc.dram_tensor('out', [N, D], F32, kind='ExternalOutput')

    with tile.TileContext(nc) as tc:
        with tc.tile_pool(name='cst', bufs=1) as cst, \
             tc.tile_pool(name='big', bufs=1) as big, \
             tc.tile_pool(name='work', bufs=3) as work, \
             tc.tile_pool(name='et', bufs=3) as etp, \
             tc.tile_pool(name='ps3', bufs=2, space='PSUM') as ps3, \
             tc.tile_pool(name='ps', bufs=2, space='PSUM') as ps:

            # ---- DMA order: wq + fhT chunks first so PE starts early ----
            t_wq = cst.tile([128, 4, HPC * HD], F32R)
            nc.sync.dma_start(t_wq[:], wq[:].rearrange('(o p) n -> p o n', p=128))
            t_fhT = big.tile([128, 4, N], F32R)
            fhT_r = fhT[:].rearrange('(o p) n -> p o n', p=128)
            for kk in range(4):
                nc.sync.dma_start(t_fhT[:, kk, :], fhT_r[:, kk, :])
            t_wk = cst.tile([128, 4, HPC * HD], F32R)
            nc.sync.dma_start(t_wk[:], wk[:].rearrange('(o p) n -> p o n', p=128))
            t_wv = cst.tile([128, 4, HPC * HD], F32R)
            nc.sync.dma_start(t_wv[:], wv[:].rearrange('(o p) n -> p o n', p=128))
            t_foT = cst.tile([2, N], F32R)
            nc.sync.dma_start(t_foT[:], foT[:])
            t_woq = cst.tile([2, 128], F32R)
            nc.sync.dma_start(t_woq[:], woq[:])
            t_wok = cst.tile([2, 128], F32R)
            nc.sync.dma_start(t_wok[:], wok[:])
            t_wo = cst.tile([128, 2, D], F32R)
            nc.sync.dma_start(t_wo[:], wo[:].rearrange('o p n -> p o n'))
            if with_bias:
                t_bq = cst.tile([1, HPC * HD], F32R)
                nc.sync.dma_start(t_bq[:], bqr[:])
                t_bk = cst.tile([1, HPC * HD], F32R)
                nc.sync.dma_start(t_bk[:], bkr[:])
                t_bv = cst.tile([1, HPC * HD], F32R)
                nc.sync.dma_start(t_bv[:], bvr[:])
                t_boq = cst.tile([1, 128], F32R)
                nc.sync.dma_start(t_boq[:], boqr[:])
                t_bok = cst.tile([1, 128], F32R)
                nc.sync.dma_start(t_bok[:], bokr[:])
                t_ones = cst.tile([1, 512], F32R)
                nc.sync.dma_start(t_ones[:], onesd[:])

            # score packs per head [128, N]; static rows DMA'd from tables
            t_qp = [big.tile([128, N], F32R, tag=f'qp{hh}', name=f'qp{hh}')
                    for hh in range(HPC)]
            t_kp = [big.tile([128, N], F32R, tag=f'kp{hh}', name=f'kp{hh}')
                    for hh in range(HPC)]
            for hh in range(HPC):
                nc.sync.dma_start(t_qp[hh][80:128, :], qstat[hh])
                nc.sync.dma_start(t_kp[hh][80:112, :], kstat[:])
            # v in natural layout per head + 64 ones columns for denominators
            t_v = [big.tile([128, KC, 128], F32R, tag=f'v{hh}', name=f'v{hh}')
                   for hh in range(HPC)]
            for hh in range(HPC):
                nc.sync.dma_start(
                    t_v[hh][:, :, 64:128],
                    vones[:, None, :].to_broadcast((128, KC, 64)))
            # attention-out^T pairs (heads 2p, 2p+1 stacked on partitions)
            t_ot = [big.tile([128, N], F32R, tag=f'ot{p}', name=f'ot{p}')
                    for p in range(2)]

            # ---- stage 1: projections ----
            # qT / kT: psum rows = 128 output channels (2 heads), cols = tokens
            # m=0 (heads 0,1) first so stage 2 can begin before m=1 finishes
            def emit_qk(m):
                for (w_t, b_name, pack, sc) in ((t_wq, 'bq', t_qp, SCALE),
                                                (t_wk, 'bk', t_kp, 1.0)):
                    for j in range(QC):
                        p_qt = ps.tile([128, 512], F32, tag='mm', name='p_qt')
                        for kk in range(4):
                            nc.tensor.matmul(
                                p_qt[:], w_t[:, kk, m * 128:(m + 1) * 128],
                                t_fhT[:, kk, j * 512:(j + 1) * 512],
                                start=(kk == 0),
                                stop=(not with_bias and kk == 3))
                        if with_bias:
                            bt = t_bq if b_name == 'bq' else t_bk
                            nc.tensor.matmul(
                                p_qt[:], bt[:, m * 128:(m + 1) * 128],
                                t_ones[:], start=False, stop=True)
                        for s in range(2):
                            hh = 2 * m + s
                            if sc == 1.0:
                                nc.scalar.copy(
                                    pack[hh][0:64, j * 512:(j + 1) * 512],
                                    p_qt[s * 64:(s + 1) * 64, :])
                            else:
                                nc.vector.tensor_scalar_mul(
                                    pack[hh][0:64, j * 512:(j + 1) * 512],
                                    p_qt[s * 64:(s + 1) * 64, :], sc)

            emit_qk(0)
            # oqT / okT: heads padded to 32-row psum boundaries
            for (w_t, b_name, pack, sc) in ((t_woq, 'boq', t_qp, OBS_SCALE),
                                            (t_wok, 'bok', t_kp, 1.0)):
                for j in range(QC):
                    p_o = ps.tile([128, 512], F32, tag='mm', name='p_o')
                    nc.tensor.matmul(p_o[:], w_t[:],
                                     t_foT[:, j * 512:(j + 1) * 512],
                                     start=True, stop=(not with_bias))
                    if with_bias:
                        bt = t_boq if b_name == 'boq' else t_bok
                        nc.tensor.matmul(p_o[:], bt[:], t_ones[:],
                                         start=False, stop=True)
                    for hh in range(HPC):
                        if sc == 1.0:
                            nc.scalar.copy(
                                pack[hh][64:80, j * 512:(j + 1) * 512],
                                p_o[hh * 32:hh * 32 + OD, :])
                        else:
                            nc.vector.tensor_scalar_mul(
                                pack[hh][64:80, j * 512:(j + 1) * 512],
                                p_o[hh * 32:hh * 32 + OD, :], sc)
            # v natural layout: psum [128 tokens, 256 channels] per token chunk
            for kc in range(KC):
                p_v = ps.tile([128, HPC * HD], F32, tag='mm', name='p_v')
                for kk in range(4):
                    nc.tensor.matmul(p_v[:], t_fhT[:, kk, kc * 128:(kc + 1) * 128],
                                     t_wv[:, kk, :], start=(kk == 0),
                                     stop=(not with_bias and kk == 3))
                if with_bias:
                    nc.tensor.matmul(p_v[:], t_ones[:, 0:128], t_bv[:],
                                     start=False, stop=True)
                for hh in range(HPC):
                    nc.vector.tensor_copy(t_v[hh][:, kc, 0:64],
                                          p_v[:, hh * 64:(hh + 1) * 64])
            emit_qk(1)

            # ---- stages 2+3+4, interleaved per q-chunk ----
            for j in range(QC):
                for hh in range(HPC):
                    # time-bias basis rows for this (head, q-chunk)
                    nc.sync.dma_start(t_kp[hh][112:128, :], apack[hh, j])
                    p_ot = ps.tile([128, 512], F32, tag='mm', name='p_ot')
                    for g in range(KC // 3):
                        p_s3 = ps3.tile([128, 3, 512], F32, tag='s3',
                                        name='p_s3')
                        for i3 in range(3):
                            kc = 3 * g + i3
                            nc.tensor.matmul(
                                p_s3[:, i3, :],
                                t_kp[hh][:, kc * 128:(kc + 1) * 128],
                                t_qp[hh][:, j * 512:(j + 1) * 512],
                                start=True, stop=True)
                        t_et = etp.tile([128, 3, 512], F32R, tag='et',
                                        name='t_et')
                        nc.scalar.activation(t_et[:], p_s3[:], EXP)
                        for i3 in range(3):
                            kc = 3 * g + i3
                            nc.tensor.matmul(p_ot[:], t_v[hh][:, kc, :],
                                             t_et[:, i3, :],
                                             start=(kc == 0),
                                             stop=(kc == KC - 1))
                    t_rec = work.tile([64, 512], F32, tag='rec', name='t_rec')
                    nc.vector.reciprocal(t_rec[:], p_ot[64:128, :])
                    nc.vector.tensor_mul(
                        t_ot[hh // 2][(hh % 2) * 64:(hh % 2) * 64 + 64,
                                      j * 512:(j + 1) * 512],
                        p_ot[0:64, :], t_rec[:])
                # partial out-projection for this q-chunk's 4 row blocks
                for qq in range(4):
                    qc = 4 * j + qq
                    p_y = ps.tile([128, D], F32, tag='mm', name='p_y')
                    for p in range(2):
                        nc.tensor.matmul(p_y[:],
                                         t_ot[p][:, qc * 128:(qc + 1) * 128],
                                         t_wo[:, p, :], start=(p == 0),
                                         stop=(p == 1))
                    t_y = work.tile([128, D], F32, tag='y', name='t_y')
                    nc.vector.tensor_copy(t_y[:], p_y[:])
                    nc.sync.dma_start(out[qc * 128:(qc + 1) * 128, :], t_y[:])

    _split_waits(nc)
    return nc


_NC_CACHE = {}


def _get_nc(with_bias=False):
    if with_bias not in _NC_CACHE:
        _NC_CACHE[with_bias] = _build(with_bias)
    return _NC_CACHE[with_bias]


def _pad_obs(a):
    # lay each head's 16 obs channels at a 32-column boundary (PSUM reads
    # must start at 32-partition-aligned offsets)
    out = np.zeros((a.shape[0], 128), np.float32)
    for hh in range(HPC):
        out[:, hh * 32:hh * 32 + OD] = a[:, hh * OD:(hh + 1) * OD]
    return out


def _host_prep(h, observation_state, Wq, bq, Wk, bk, Wv, bv, Wo, bo,
               Woq, boq, Wok, bok, variable_bias, relative_time_bias,
               with_bias=False):
    f32 = np.float32
    h = np.asarray(h, f32)
    obs = np.asarray(observation_state, f32)
    Kidx = np.arange(N)
    tK = Kidx // V                                     # time bin of each token
    kstat = (Kidx[None, :] % V == np.arange(V)[:, None]).astype(f32)
    bq16 = ((Kidx[None, :] // V) % 16 == np.arange(16)[:, None]).astype(f32)

    in_maps = []
    for c in range(NCORES):
        b, hg = divmod(c, 2)
        h0 = hg * HPC
        cs, ce = h0 * HD, (h0 + HPC) * HD
        os_, oe = h0 * OD, (h0 + HPC) * OD
        qstat = np.empty((HPC, 48, N), f32)
        ap = np.empty((HPC, QC, 16, N), f32)
        for hh in range(HPC):
            head = h0 + hh
            vb = np.asarray(variable_bias[head], f32)
            rtb = np.asarray(relative_time_bias[head], f32)
            qstat[hh, :V] = vb[Kidx % V, :].T          # VB_h[Q%32, j]
            qstat[hh, V:] = bq16
            for j in range(QC):
                # A_hj[s, K] = rtb[16j + s - K//32 + 47]
                idx = 16 * j + np.arange(16)[:, None] - tK[None, :] + (T - 1)
                ap[hh, j] = rtb[idx]
        m = {
            'fhT': np.ascontiguousarray(h[b].reshape(N, D).T),
            'foT': np.ascontiguousarray(obs[b].reshape(N, 2).T),
            'wq': np.ascontiguousarray(np.asarray(Wq, f32)[:, cs:ce]),
            'wk': np.ascontiguousarray(np.asarray(Wk, f32)[:, cs:ce]),
            'wv': np.ascontiguousarray(np.asarray(Wv, f32)[:, cs:ce]),
            'woq': _pad_obs(np.asarray(Woq, f32)[:, os_:oe]),
            'wok': _pad_obs(np.asarray(Wok, f32)[:, os_:oe]),
            'wo': np.ascontiguousarray(
                np.asarray(Wo, f32)[cs:ce, :].reshape(2, 128, D)),
            'kstat': kstat,
            'qstat': qstat,
            'apack': ap,
            'vones': np.ones((128, 64), f32),
        }
        if with_bias:
            m.update({
                'bqr': np.ascontiguousarray(np.asarray(bq, f32)[None, cs:ce]),
                'bkr': np.ascontiguousarray(np.asarray(bk, f32)[None, cs:ce]),
                'bvr': np.ascontiguousarray(np.asarray(bv, f32)[None, cs:ce]),
                'boqr': _pad_obs(np.asarray(boq, f32)[None, os_:oe]),
                'bokr': _pad_obs(np.asarray(bok, f32)[None, os_:oe]),
                'onesd': np.ones((1, 512), f32),
            })
        in_maps.append(m)
    return in_maps


def kernel(**inputs):
    with_bias = any(
        np.any(np.asarray(inputs[k])) for k in ('bq', 'bk', 'bv', 'boq', 'bok'))
    nc = _get_nc(with_bias)
    in_maps = _host_prep(**inputs, with_bias=with_bias)
    res = run_bass_kernel_spmd(nc, in_maps, core_ids=list(range(NCORES)))
    bo = np.asarray(inputs['bo'], np.float32)
    outf = np.zeros((B, N, D), np.float32)
    for c in range(NCORES):
        outf[c // 2] += res.results[c]['out']
    outf += bo[None, None, :]
    return outf.reshape(B, T, V, D)



# revision 4
# speedup vs baseline: 1.1680x; 1.1680x over previous
"""Trainium2 Bass kernel for ClinicalStateFormationOperator.

Full-input contract: kernel(**inputs) takes the complete (unsharded) numpy
inputs and returns the full [B, T, V, D] output. Internally the work is
sharded across 8 NeuronCores as (batch, head-group): core c handles batch
c//2 and heads (c%2)*4 .. (c%2)*4+3. Each core computes its 4 heads'
attention and the partial output projection; the host sums the two partial
projections per batch and adds the output bias.

v2 design (vs the v1 baseline, 143.9us):
 - Engine rebalance: Activation runs ONLY the 48 softmax exps (its cost-model
   floor, ~73us); all psum->sbuf copies live on DVE; obs-state projections
   (K=2 matmuls) move to host prep entirely.
 - All operand tiles are bf16 (same PE rate as float32r in the cost model,
   half the DMA/SBUF): packs, E=exp(scores), v, attention-out, weights.
   Measured end-to-end rel err ~8e-3 vs the 2e-2 gate.
 - Software-pipelined schedule: score matmuls for quad r and the AV matmuls
   for quad r-1 interleave on PE with stage-1 projections/out-projections
   drip-fed as fillers, so PE (the 82us bound) never stalls on ACT.
 - PSUM: 2x[128,3,512] score groups (6 banks) + 1 AV accumulator + 1
   proj/outproj bank = 8 banks.

Per-quad math (quad = (head h, 512-query chunk j), N = T*V = 1536 tokens):
scores are computed transposed (keys on partitions, queries free) in ONE
K=128 matmul per [128k x 512q] tile by packing four contraction groups into
the 128 pack rows:
    rows  0: 64  kT_h          |  qT_h            (content; sqrt(scale)
                                                   folded into Wq AND Wk)
    rows 64: 80  okT_h         |  oqT_h           (observation, host-computed
                                                   with sqrt(obs_scale) folded)
    rows 80:112  [K%32==r]     |  VB_h[Q%32, r]   (variable bias)
    rows112:128  A_hj[s,K]=rtb_h[16j+s-K//32+47] | [(Q//32)%16==s]  (time
                 bias; A rows re-DMA'd into the k-pack once per (h, j),
                 prefetched a full j-round ahead)
    E^T = exp(scores^T) in bf16  (|scores| <~ 6, fp32 psum in, no max-sub)
    [out^T; denom_rep] = [v_h | ones]^T @ E^T  (64 ones columns replicate
         the softmax denominator -> aligned DVE divide)
    OT = out^T * reciprocal(denom_rep)         (bf16)
    y_partial = OT^T_headpairs @ Wo_rows       (host sums core pairs + bo)
"""

from collections import deque

import numpy as np
import ml_dtypes

import concourse.bass as bass
import concourse.mybir as mybir
import concourse.tile as tile
from concourse.bass_utils import run_bass_kernel_spmd

V = 32
T = 48
D = 512
H = 8
HD = D // H          # 64
OD = 16
B = 4
N = T * V            # 1536
HPC = 4              # heads per core
NCORES = 8
SCALE = 1.0 / np.sqrt(HD)
OBS_SCALE = 1.0 / np.sqrt(OD)

F32 = mybir.dt.float32
BF16 = mybir.dt.bfloat16
NPBF = ml_dtypes.bfloat16
EXP = mybir.ActivationFunctionType.Exp

KC = N // 128        # 12 key chunks of 128
QC = N // 512        # 3 query chunks of 512
NR = HPC * QC        # 12 quads (rounds)


def _split_waits(nc, max_waits=1):
    """Walrus in this container allows only one sync-wait slot per
    instruction; spill extra waits onto preceding same-engine NoOps."""
    def fix_bb(bb):
        changed = False
        new = []
        for inst in bb.instructions:
            si = inst.sync_info
            if si is not None and len(si.on_wait) > max_waits:
                waits = list(si.on_wait)
                for w in waits[:-max_waits]:
                    new.append(mybir.InstNoOp(
                        name=nc.get_next_instruction_name(),
                        engine=inst.engine, ins=[], outs=[],
                        sync_info=mybir.SyncInfo(on_wait=[w], on_update=[])))
                    changed = True
                si.on_wait = waits[-max_waits:]
            new.append(inst)
        if changed:
            bb.instructions = new
        for sub in getattr(bb, 'blocks', []) or []:
            fix_bb(sub)
    for f in nc.m.functions:
        for bb in f.blocks:
            fix_bb(bb)


def _build(with_bias=False):
    nc = bass.Bass()

    # ---- per-core DRAM I/O (data differs per core, program is SPMD) ----
    fhT = nc.dram_tensor('fhT', [D, N], BF16, kind='ExternalInput')
    wq = nc.dram_tensor('wq', [D, HPC * HD], BF16, kind='ExternalInput')
    wk = nc.dram_tensor('wk', [D, HPC * HD], BF16, kind='ExternalInput')
    wv = nc.dram_tensor('wv', [D, HPC * HD], BF16, kind='ExternalInput')
    wo = nc.dram_tensor('wo', [2, 128, D], BF16, kind='ExternalInput')
    # static pack rows (host-built): qtab = [obs-q 16 | var-values 32 |
    # time-indicator 16] rows, ktab = [obs-k 16 | var-indicator 32]
    qtab = nc.dram_tensor('qtab', [HPC, 64, N], BF16, kind='ExternalInput')
    ktab = nc.dram_tensor('ktab', [HPC, 48, N], BF16, kind='ExternalInput')
    atab = nc.dram_tensor('atab', [HPC, QC, 16, N], BF16,
                          kind='ExternalInput')
    vones = nc.dram_tensor('vones', [128, 64], BF16, kind='ExternalInput')
    if with_bias:
        bqr = nc.dram_tensor('bqr', [1, HPC * HD], BF16, kind='ExternalInput')
        bkr = nc.dram_tensor('bkr', [1, HPC * HD], BF16, kind='ExternalInput')
        bvr = nc.dram_tensor('bvr', [1, HPC * HD], BF16, kind='ExternalInput')
        onesd = nc.dram_tensor('onesd', [1, 512], BF16, kind='ExternalInput')
    out = nc.dram_tensor('out', [N, D], F32, kind='ExternalOutput')

    with tile.TileContext(nc) as tc:
        with tc.tile_pool(name='sb', bufs=1) as sb, \
             tc.tile_pool(name='etp', bufs=8) as etp, \
             tc.tile_pool(name='wkp', bufs=2) as wkp, \
             tc.tile_pool(name='psp', bufs=1, space='PSUM') as psp:

            t_fhT = sb.tile([128, 4, N], BF16)
            t_wq = sb.tile([128, 4, HPC * HD], BF16)
            t_wk = sb.tile([128, 4, HPC * HD], BF16)
            t_wv = sb.tile([128, 4, HPC * HD], BF16)
            t_wo = sb.tile([128, 2, D], BF16)
            t_qp = [sb.tile([128, N], BF16, name=f'qp{h}') for h in range(HPC)]
            t_kp = [sb.tile([128, N], BF16, name=f'kp{h}') for h in range(HPC)]
            # v packs: [keys, kc, head, 64 v-ch | 64 ones]
            v4 = sb.tile([128, KC, HPC, 128], BF16)
            t_ot = [sb.tile([128, N], BF16, name=f'ot{p}') for p in range(2)]
            if with_bias:
                t_bq = sb.tile([1, HPC * HD], BF16)
                t_bk = sb.tile([1, HPC * HD], BF16)
                t_bv = sb.tile([1, HPC * HD], BF16)
                t_ones = sb.tile([1, 512], BF16)

            # ---- DMA order: first-needed first ----
            nc.sync.dma_start(t_wq[:], wq[:].rearrange('(o p) n -> p o n', p=128))
            fhT_r = fhT[:].rearrange('(o p) n -> p o n', p=128)
            for kk in range(4):
                nc.sync.dma_start(t_fhT[:, kk, 0:512], fhT_r[:, kk, 0:512])
            nc.sync.dma_start(t_wk[:], wk[:].rearrange('(o p) n -> p o n', p=128))
            for h in range(HPC):
                nc.sync.dma_start(t_qp[h][64:128, :], qtab[h])
                nc.sync.dma_start(t_kp[h][64:112, :], ktab[h])
                nc.sync.dma_start(t_kp[h][112:128, :], atab[h, 0])
            nc.sync.dma_start(t_wv[:], wv[:].rearrange('(o p) n -> p o n', p=128))
            if with_bias:
                nc.sync.dma_start(t_bq[:], bqr[:])
                nc.sync.dma_start(t_bk[:], bkr[:])
                nc.sync.dma_start(t_bv[:], bvr[:])
                nc.sync.dma_start(t_ones[:], onesd[:])
            for j in range(1, QC):
                for kk in range(4):
                    nc.sync.dma_start(t_fhT[:, kk, j * 512:(j + 1) * 512],
                                      fhT_r[:, kk, j * 512:(j + 1) * 512])
            nc.sync.dma_start(t_wo[:], wo[:].rearrange('o p n -> p o n'))
            for kc in range(KC):
                nc.sync.dma_start(
                    v4[:, kc, :, 64:128],
                    vones[:, None, :].to_broadcast((128, HPC, 64)))

            # ---- stage-1 emitters (run as fillers inside the quad loop) ----
            def emit_q(m, j, w_t, b_t, packs, nm):
                p = psp.tile([128, 512], F32, tag='mm', bufs=1,
                             name=f'p_{nm}_{m}{j}')
                for kk in range(4):
                    nc.tensor.matmul(
                        p[:], w_t[:, kk, m * 128:(m + 1) * 128],
                        t_fhT[:, kk, j * 512:(j + 1) * 512],
                        start=(kk == 0), stop=(not with_bias and kk == 3))
                if with_bias:
                    nc.tensor.matmul(p[:], b_t[:, m * 128:(m + 1) * 128],
                                     t_ones[:], start=False, stop=True)
                for s in range(2):
                    nc.vector.tensor_copy(
                        packs[2 * m + s][0:64, j * 512:(j + 1) * 512],
                        p[s * 64:(s + 1) * 64, :])

            def emit_v(kc):
                p = psp.tile([128, HPC * HD], F32, tag='mm', bufs=1,
                             name=f'p_v{kc}')
                for kk in range(4):
                    nc.tensor.matmul(p[:], t_fhT[:, kk, kc * 128:(kc + 1) * 128],
                                     t_wv[:, kk, :], start=(kk == 0),
                                     stop=(not with_bias and kk == 3))
                if with_bias:
                    nc.tensor.matmul(p[:], t_ones[:, 0:128], t_bv[:],
                                     start=False, stop=True)
                nc.vector.tensor_copy(v4[:, kc, :, 0:64], p[:])

            def emit_outproj(j, qq):
                qc = 4 * j + qq
                p = psp.tile([128, D], F32, tag='mm', bufs=1, name=f'p_y{qc}')
                for pp in range(2):
                    nc.tensor.matmul(p[:], t_ot[pp][:, qc * 128:(qc + 1) * 128],
                                     t_wo[:, pp, :], start=(pp == 0),
                                     stop=(pp == 1))
                t_y = wkp.tile([128, D], F32, tag='y', name=f't_y{qc}')
                nc.vector.tensor_copy(t_y[:], p[:])
                nc.sync.dma_start(out[qc * 128:(qc + 1) * 128, :], t_y[:])

            fillers = deque()

            def fill(n):
                for _ in range(min(n, len(fillers))):
                    fillers.popleft()()

            def Q(m, j):
                return lambda: emit_q(m, j, t_wq, t_bq if with_bias else None,
                                      t_qp, 'q')

            def K(m, j):
                return lambda: emit_q(m, j, t_wk, t_bk if with_bias else None,
                                      t_kp, 'k')

            fillers.extend(
                [K(0, 1), K(0, 2)]
                + [lambda kc=kc: emit_v(kc) for kc in range(3)]
                + [Q(1, 0), K(1, 0)]
                + [lambda kc=kc: emit_v(kc) for kc in range(3, 9)]
                + [lambda kc=kc: emit_v(kc) for kc in range(9, 12)]
                + [K(1, 1), K(1, 2), Q(0, 1), Q(1, 1), Q(0, 2), Q(1, 2)])

            # ---- software-pipelined quad rounds ----
            ets = {}
            FILLS = {0: (2, 2, 1, 1), 1: (2, 2, 2, 1), 2: (2, 2, 1, 1)}

            def emit_sc(r):
                j, h = r // HPC, r % HPC
                budget = FILLS.get(r, (1, 1, 1, 1))
                lst = []
                for g in range(4):
                    p_s3 = psp.tile([128, 3, 512], F32, tag='s3', bufs=2,
                                    name=f'p_s3_{r}_{g}')
                    for i3 in range(3):
                        kc = 3 * g + i3
                        nc.tensor.matmul(
                            p_s3[:, i3, :],
                            t_kp[h][:, kc * 128:(kc + 1) * 128],
                            t_qp[h][:, j * 512:(j + 1) * 512],
                            start=True, stop=True)
                    et = etp.tile([128, 3, 512], BF16, tag='et',
                                  name=f'et_{r}_{g}')
                    nc.scalar.activation(et[:], p_s3[:], EXP)
                    lst.append(et)
                    fill(budget[g])
                ets[r] = lst
                if j + 1 < QC:  # prefetch next j-round's time-bias rows
                    nc.sync.dma_start(t_kp[h][112:128, :], atab[h, j + 1])

            def emit_av(r):
                j, h = r // HPC, r % HPC
                p_av = psp.tile([128, 512], F32, tag='av', bufs=1,
                                name=f'p_av_{r}')
                lst = ets.pop(r)
                for kc in range(KC):
                    if kc in (3, 6, 9):
                        fill(1)
                    nc.tensor.matmul(p_av[:], v4[:, kc, h, :],
                                     lst[kc // 3][:, kc % 3, :],
                                     start=(kc == 0), stop=(kc == KC - 1))
                rec = wkp.tile([64, 512], F32, tag='rec', name=f'rec_{r}')
                nc.vector.reciprocal(rec[:], p_av[64:128, :])
                nc.vector.tensor_mul(
                    t_ot[h // 2][(h % 2) * 64:(h % 2) * 64 + 64,
                                 j * 512:(j + 1) * 512],
                    p_av[0:64, :], rec[:])
                if h == HPC - 1:  # whole j-column normalized -> out-projection
                    fillers.extend(
                        [lambda qq=qq, j=j: emit_outproj(j, qq)
                         for qq in range(4)])

            emit_q(0, 0, t_wq, t_bq if with_bias else None, t_qp, 'q')
            emit_q(0, 0, t_wk, t_bk if with_bias else None, t_kp, 'k')
            for r in range(NR):
                emit_sc(r)
                if r >= 1:
                    emit_av(r - 1)
            emit_av(NR - 1)
            fill(len(fillers))

    _split_waits(nc)
    return nc


_NC_CACHE = {}


def _get_nc(with_bias=False):
    if with_bias not in _NC_CACHE:
        _NC_CACHE[with_bias] = _build(with_bias)
    return _NC_CACHE[with_bias]


def _host_prep(h, observation_state, Wq, bq, Wk, bk, Wv, bv, Wo, bo,
               Woq, boq, Wok, bok, variable_bias, relative_time_bias,
               with_bias=False):
    f32 = np.float32
    h = np.asarray(h, f32)
    obs = np.asarray(observation_state, f32).reshape(B, N, 2)
    Kidx = np.arange(N)
    tK = Kidx // V                                 # time bin of each token
    sq = np.float32(np.sqrt(SCALE))
    so = np.float32(np.sqrt(OBS_SCALE))
    kvar = (Kidx[None, :] % V == np.arange(V)[:, None]).astype(f32)  # [32,N]
    bq16 = ((Kidx[None, :] // V) % 16 == np.arange(16)[:, None]).astype(f32)

    # host obs projections (K=2 matmuls), sqrt(obs_scale) + bias folded
    Woq_s = np.asarray(Woq, f32) * so
    Wok_s = np.asarray(Wok, f32) * so
    oq = obs @ Woq_s + np.asarray(boq, f32) * so       # [B, N, 128]
    ok = obs @ Wok_s + np.asarray(bok, f32) * so

    Wq_s = np.asarray(Wq, f32) * sq
    Wk_s = np.asarray(Wk, f32) * sq

    in_maps = []
    for c in range(NCORES):
        b, hg = divmod(c, 2)
        h0 = hg * HPC
        cs, ce = h0 * HD, (h0 + HPC) * HD
        qt = np.empty((HPC, 64, N), f32)
        kt = np.empty((HPC, 48, N), f32)
        at = np.empty((HPC, QC, 16, N), f32)
        for hh in range(HPC):
            head = h0 + hh
            vb = np.asarray(variable_bias[head], f32)
            rtb = np.asarray(relative_time_bias[head], f32)
            qt[hh, 0:16] = oq[b, :, head * OD:(head + 1) * OD].T
            qt[hh, 16:48] = vb[Kidx % V, :].T          # VB_h[Q%32, r]
            qt[hh, 48:64] = bq16
            kt[hh, 0:16] = ok[b, :, head * OD:(head + 1) * OD].T
            kt[hh, 16:48] = kvar
            for j in range(QC):
                # A_hj[s, K] = rtb[16j + s - K//32 + 47]
                idx = 16 * j + np.arange(16)[:, None] - tK[None, :] + (T - 1)
                at[hh, j] = rtb[idx]
        m = {
            'fhT': np.ascontiguousarray(h[b].reshape(N, D).T).astype(NPBF),
            'wq': np.ascontiguousarray(Wq_s[:, cs:ce]).astype(NPBF),
            'wk': np.ascontiguousarray(Wk_s[:, cs:ce]).astype(NPBF),
            'wv': np.ascontiguousarray(np.asarray(Wv, f32)[:, cs:ce]).astype(NPBF),
            'wo': np.ascontiguousarray(
                np.asarray(Wo, f32)[cs:ce, :].reshape(2, 128, D)).astype(NPBF),
            'qtab': qt.astype(NPBF),
            'ktab': kt.astype(NPBF),
            'atab': at.astype(NPBF),
            'vones': np.ones((128, 64), NPBF),
        }
        if with_bias:
            m.update({
                'bqr': (np.asarray(bq, f32)[None, cs:ce] * sq).astype(NPBF),
                'bkr': (np.asarray(bk, f32)[None, cs:ce] * sq).astype(NPBF),
                'bvr': np.asarray(bv, f32)[None, cs:ce].astype(NPBF),
                'onesd': np.ones((1, 512), NPBF),
            })
        in_maps.append(m)
    return in_maps


def kernel(**inputs):
    with_bias = any(
        np.any(np.asarray(inputs[k])) for k in ('bq', 'bk', 'bv'))
    nc = _get_nc(with_bias)
    in_maps = _host_prep(**inputs, with_bias=with_bias)
    res = run_bass_kernel_spmd(nc, in_maps, core_ids=list(range(NCORES)))
    bo = np.asarray(inputs['bo'], np.float32)
    outf = np.zeros((B, N, D), np.float32)
    for c in range(NCORES):
        outf[c // 2] += res.results[c]['out']
    outf += bo[None, None, :]
    return outf.reshape(B, T, V, D)


# revision 28
# speedup vs baseline: 1.4184x; 1.2144x over previous
"""Trainium2 Bass kernel for ClinicalStateFormationOperator.

Full-input contract: kernel(**inputs) takes the complete (unsharded) numpy
inputs and returns the full [B, T, V, D] output. Internally the work is
sharded across 8 NeuronCores as (batch, head-group): core c handles batch
c//2 and heads (c%2)*4 .. (c%2)*4+3. Each core computes its 4 heads'
attention and the partial output projection; the host sums the two partial
projections per batch and adds the output bias.

v5 design (v1 baseline 143.9us -> 101.5us cost-model time; rel err 8.8e-3):
 - Engine rebalance: Activation runs ONLY the 48 softmax exps (its cost-model
   floor, ~73us); psum->sbuf copies live on DVE; obs-state projections (K=2
   matmuls) are host prep; Pool/gpsimd cannot touch PSUM so it idles.
 - All operand tiles are bf16 (same PE rate as float32r in the cost model,
   half the DMA/SBUF): packs, E=exp(scores), v, attention-out, weights.
   Measured end-to-end rel err ~7.7e-3 vs the 2e-2 gate.
 - Software pipeline: round r emits the score matmuls + exps of quad r and
   (per the AVS table) the AV matmuls of a quad 2-3 rounds back. Stage-1
   projection / out-projection tasks drip from a deadline-guarded queue;
   consecutive fillers alternate between the 'mm' and (while free,
   rounds < 3) 'av' psum banks so each filler's psum->pack DVE copy
   overlaps the next filler's matmuls instead of stalling PE on the
   bank's write-after-read.
 - PSUM: 2x[128,3,512] score groups (6 banks) + 1 AV accumulator + 1
   proj/outproj bank = 8. The prefix projections and the tail
   out-projections borrow the av/s3 banks, which are idle at those times.
 - Weights/activations are DMA'd in device layout (host pre-transposed),
   first-needed first, split across the SP and ACT HWDGE queues.

Per-quad math (quad = (head h, 512-query chunk j), N = T*V = 1536 tokens):
scores are computed transposed (keys on partitions, queries free) in ONE
K=128 matmul per [128k x 512q] tile by packing four contraction groups into
the 128 pack rows:
    rows  0: 64  kT_h          |  qT_h            (content; sqrt(scale)
                                                   folded into Wq AND Wk)
    rows 64: 80  okT_h         |  oqT_h           (observation, host-computed
                                                   with sqrt(obs_scale) folded)
    rows 80:112  [K%32==r]     |  VB_h[Q%32, r]   (variable bias)
    rows112:128  A_hj[s,K]=rtb_h[16j+s-K//32+47] | [(Q//32)%16==s]  (time
                 bias; A rows re-DMA'd into the k-pack once per (h, j),
                 prefetched a full j-round ahead)
    E^T = exp(scores^T) in bf16  (|scores| <~ 6, fp32 psum in, no max-sub)
    [out^T; denom_rep] = [v_h | ones]^T @ E^T  (64 ones columns replicate
         the softmax denominator -> aligned DVE divide)
    OT = out^T * reciprocal(denom_rep)         (bf16)
    y_partial = OT^T_headpairs @ Wo_rows       (host sums core pairs + bo)
"""

from collections import deque

import numpy as np
import ml_dtypes

import concourse.bass as bass
import concourse.mybir as mybir
import concourse.tile as tile
from concourse.bass_utils import run_bass_kernel_spmd

V = 32
T = 48
D = 512
H = 8
HD = D // H          # 64
OD = 16
B = 4
N = T * V            # 1536
HPC = 4              # heads per core
NCORES = 8
SCALE = 1.0 / np.sqrt(HD)
OBS_SCALE = 1.0 / np.sqrt(OD)

F32 = mybir.dt.float32
BF16 = mybir.dt.bfloat16
NPBF = ml_dtypes.bfloat16
EXP = mybir.ActivationFunctionType.Exp

KC = N // 128        # 12 key chunks of 128
QC = N // 512        # 3 query chunks of 512
NR = HPC * QC        # 12 quads (rounds)
LAG = 2              # AV trails scores by 2 rounds


def _split_waits(nc, max_waits=1):
    """Walrus in this container allows only one sync-wait slot per
    instruction; spill extra waits onto preceding same-engine NoOps."""
    def fix_bb(bb):
        changed = False
        new = []
        for inst in bb.instructions:
            si = inst.sync_info
            if si is not None and len(si.on_wait) > max_waits:
                waits = list(si.on_wait)
                for w in waits[:-max_waits]:
                    new.append(mybir.InstNoOp(
                        name=nc.get_next_instruction_name(),
                        engine=inst.engine, ins=[], outs=[],
                        sync_info=mybir.SyncInfo(on_wait=[w], on_update=[])))
                    changed = True
                si.on_wait = waits[-max_waits:]
            new.append(inst)
        if changed:
            bb.instructions = new
        for sub in getattr(bb, 'blocks', []) or []:
            fix_bb(sub)
    for f in nc.m.functions:
        for bb in f.blocks:
            fix_bb(bb)


def _build(with_bias=False):
    nc = bass.Bass()

    # ---- per-core DRAM I/O, already in device layout (host transposes) ----
    fhT = nc.dram_tensor('fhT', [128, 4, N], BF16, kind='ExternalInput')
    wq = nc.dram_tensor('wq', [128, 4, HPC * HD], BF16, kind='ExternalInput')
    wk = nc.dram_tensor('wk', [128, 4, HPC * HD], BF16, kind='ExternalInput')
    wv = nc.dram_tensor('wv', [128, 4, HPC * HD], BF16, kind='ExternalInput')
    wo = nc.dram_tensor('wo', [128, 2, D], BF16, kind='ExternalInput')
    # static pack rows (host-built): qtab = [obs-q 16 | var-values 32 |
    # time-indicator 16] rows, ktab = [obs-k 16 | var-indicator 32]
    qtab = nc.dram_tensor('qtab', [HPC, 64, N], BF16, kind='ExternalInput')
    ktab = nc.dram_tensor('ktab', [HPC, 48, N], BF16, kind='ExternalInput')
    atab = nc.dram_tensor('atab', [HPC, QC, 16, N], BF16,
                          kind='ExternalInput')
    vones = nc.dram_tensor('vones', [128, 64], BF16, kind='ExternalInput')
    if with_bias:
        bqr = nc.dram_tensor('bqr', [1, HPC * HD], BF16, kind='ExternalInput')
        bkr = nc.dram_tensor('bkr', [1, HPC * HD], BF16, kind='ExternalInput')
        bvr = nc.dram_tensor('bvr', [1, HPC * HD], BF16, kind='ExternalInput')
        onesd = nc.dram_tensor('onesd', [1, 512], BF16, kind='ExternalInput')
    out = nc.dram_tensor('out', [N, D], BF16, kind='ExternalOutput')

    with tile.TileContext(nc) as tc:
        with tc.tile_pool(name='sb', bufs=1) as sb, \
             tc.tile_pool(name='etp', bufs=16) as etp, \
             tc.tile_pool(name='wkp', bufs=2) as wkp, \
             tc.tile_pool(name='psp', bufs=1, space='PSUM') as psp:

            t_fhT = sb.tile([128, 4, N], BF16)
            t_wq = sb.tile([128, 4, HPC * HD], BF16)
            t_wk = sb.tile([128, 4, HPC * HD], BF16)
            t_wv = sb.tile([128, 4, HPC * HD], BF16)
            t_wo = sb.tile([128, 2, D], BF16)
            t_qp = [sb.tile([128, N], BF16, name=f'qp{h}') for h in range(HPC)]
            t_kp = [sb.tile([128, N], BF16, name=f'kp{h}') for h in range(HPC)]
            # v packs: [keys, kc, head, 64 v-ch | 64 ones]
            v4 = sb.tile([128, KC, HPC, 128], BF16)
            t_ot = [sb.tile([128, N], BF16, name=f'ot{p}') for p in range(2)]
            if with_bias:
                t_bq = sb.tile([1, HPC * HD], BF16)
                t_bk = sb.tile([1, HPC * HD], BF16)
                t_bv = sb.tile([1, HPC * HD], BF16)
                t_ones = sb.tile([1, 512], BF16)

            # ---- critical DMAs only; the rest are emitted post-prefix ----
            nc.scalar.dma_start(t_wq[:], wq[:])
            for kk in range(4):
                eng = nc.sync if kk % 2 == 0 else nc.scalar
                eng.dma_start(t_fhT[:, kk, 0:512], fhT[:, kk, 0:512])
            nc.sync.dma_start(t_wk[:], wk[:])
            nc.sync.dma_start(t_kp[0][64:112, :], ktab[0])
            nc.sync.dma_start(t_qp[0][64:128, :], qtab[0])
            nc.sync.dma_start(t_kp[0][112:128, :], atab[0, 0])
            nc.scalar.dma_start(t_wv[:], wv[:])

            def emit_rest_dmas():
                # fhT j1/j2 feed the K(0,1)/K(0,2) fillers popped ~10-13us in
                for j in range(1, QC):
                    for kk in range(4):
                        eng = nc.sync if kk % 2 == 0 else nc.scalar
                        eng.dma_start(t_fhT[:, kk, j * 512:(j + 1) * 512],
                                      fhT[:, kk, j * 512:(j + 1) * 512])
                for h in range(1, HPC):
                    nc.sync.dma_start(t_kp[h][64:112, :], ktab[h])
                    nc.sync.dma_start(t_qp[h][64:128, :], qtab[h])
                    nc.sync.dma_start(t_kp[h][112:128, :], atab[h, 0])
                if with_bias:
                    nc.sync.dma_start(t_bq[:], bqr[:])
                    nc.sync.dma_start(t_bk[:], bkr[:])
                    nc.sync.dma_start(t_bv[:], bvr[:])
                    nc.sync.dma_start(t_ones[:], onesd[:])
                nc.sync.dma_start(t_wo[:], wo[:])
                for kc in range(KC):
                    nc.sync.dma_start(
                        v4[:, kc, :, 64:128],
                        vones[:, None, :].to_broadcast((128, HPC, 64)))

            # ---- stage-1 emitters (run as fillers inside the quad loop) ----
            def emit_q(m, j, w_t, b_t, packs, nm, tag='mm', on_act=False):
                p = psp.tile([128, 512], F32, tag=tag,
                             bufs=2 if tag == 's3' else 1, name=f'p_{nm}_{m}{j}')
                for kk in range(4):
                    nc.tensor.matmul(
                        p[:], w_t[:, kk, m * 128:(m + 1) * 128],
                        t_fhT[:, kk, j * 512:(j + 1) * 512],
                        start=(kk == 0), stop=(not with_bias and kk == 3))
                if with_bias:
                    nc.tensor.matmul(p[:], b_t[:, m * 128:(m + 1) * 128],
                                     t_ones[:], start=False, stop=True)
                for s in range(2):
                    dst = packs[2 * m + s][0:64, j * 512:(j + 1) * 512]
                    if on_act:  # ACT is idle during warm-up; unblock DVE
                        nc.scalar.copy(dst, p[s * 64:(s + 1) * 64, :])
                    else:
                        nc.vector.tensor_copy(dst, p[s * 64:(s + 1) * 64, :])

            def emit_v(kc, tag='mm'):
                p = psp.tile([128, HPC * HD], F32, tag=tag,
                             bufs=2 if tag == 's3' else 1, name=f'p_v{kc}')
                for kk in range(4):
                    nc.tensor.matmul(p[:], t_fhT[:, kk, kc * 128:(kc + 1) * 128],
                                     t_wv[:, kk, :], start=(kk == 0),
                                     stop=(not with_bias and kk == 3))
                if with_bias:
                    nc.tensor.matmul(p[:], t_ones[:, 0:128], t_bv[:],
                                     start=False, stop=True)
                nc.vector.tensor_copy(v4[:, kc, :, 0:64], p[:])

            def emit_outproj(j, qq, tail=False):
                qc = 4 * j + qq
                tag = ('s3', 's3', 'av', 'mm')[qq] if tail else 'mm'
                p = psp.tile([128, D], F32, tag=tag,
                             bufs=2 if tag == 's3' else 1, name=f'p_y{qc}')
                for pp in range(2):
                    nc.tensor.matmul(p[:], t_ot[pp][:, qc * 128:(qc + 1) * 128],
                                     t_wo[:, pp, :], start=(pp == 0),
                                     stop=(pp == 1))
                t_y = wkp.tile([128, D], BF16, tag='y', bufs=4, name=f't_y{qc}')
                if tail and qq % 2 == 0:
                    nc.scalar.copy(t_y[:], p[:])
                else:
                    nc.vector.tensor_copy(t_y[:], p[:])
                eng = nc.scalar if (tail and qq % 2 == 0) else nc.sync
                eng.dma_start(out[qc * 128:(qc + 1) * 128, :], t_y[:])

            fillers = deque()
            late = deque()          # out-projections, drained from round 8
            state = {'mm': 0, 'next': 3.0, 'popped': 0, 'late_ok': False,
                     'r': -1}

            def tick(k=1):
                state['mm'] += k
                while state['mm'] >= state['next']:
                    if fillers:
                        fn, sp, _ = fillers.popleft()
                    elif state['late_ok'] and late:
                        fn, sp = late.popleft()
                    else:
                        break
                    fn()
                    state['next'] += sp

            def force(dl):
                # hard deadline: emit every filler due before point `dl` NOW
                while fillers and fillers[0][2] <= dl:
                    fn, sp, _ = fillers.popleft()
                    fn()
                    state['next'] += sp

            def fill_all():
                while fillers:
                    fillers.popleft()[0]()
                while late:
                    late.popleft()[0]()

            def alt_tag(i):
                # 2nd bank is free until av(0) claims it in round 3
                return 'av' if (i % 2 and state['r'] < 3) else 'mm'

            def Q(m, j):
                return lambda: emit_q(m, j, t_wq, t_bq if with_bias else None,
                                      t_qp, 'q', tag=alt_tag(j + 1))

            def K(m, j, on_act=False):
                return lambda: emit_q(m, j, t_wk, t_bk if with_bias else None,
                                      t_kp, 'k', tag=alt_tag(j),
                                      on_act=on_act)

            def Vt(kc):
                return lambda: emit_v(kc, tag=alt_tag(kc))

            # deadline-ordered: K0* before quad(j0,h0) groups; Q10/K1* before
            # quad(j0,h2); all V before av(0) at round 2; Q*1/Q*2 before j1/j2
            fillers.extend(
                [(K(0, 1), 2.4, 0), (K(0, 2), 2.4, 0), (Q(1, 0), 2.4, 2),
                 (K(1, 0), 2.4, 2), (K(1, 1), 2.4, 2),
                 (K(1, 2), 2.4, 2)]
                + [(Vt(kc), 5.5, 3.5) for kc in range(2, KC)]
                + [(Q(0, 1), 2.4, 4), (Q(1, 1), 2.4, 6), (Q(0, 2), 2.4, 8),
                   (Q(1, 2), 2.4, 10)])

            # ---- software-pipelined quad rounds ----
            ets = {}

            def emit_sc(r):
                j, h = r // HPC, r % HPC
                lst = []
                for g in range(4):
                    p_s3 = psp.tile([128, 3, 512], F32, tag='s3', bufs=2,
                                    name=f'p_s3_{r}_{g}')
                    for i3 in range(3):
                        kc = 3 * g + i3
                        nc.tensor.matmul(
                            p_s3[:, i3, :],
                            t_kp[h][:, kc * 128:(kc + 1) * 128],
                            t_qp[h][:, j * 512:(j + 1) * 512],
                            start=True, stop=True)
                        tick()
                    et = etp.tile([128, 3, 512], BF16, tag='et',
                                  name=f'et_{r}_{g}')
                    nc.scalar.activation(et[:], p_s3[:], EXP)
                    lst.append(et)
                ets[r] = lst
                if j + 1 < QC:  # prefetch next j-round's time-bias rows
                    nc.sync.dma_start(t_kp[h][112:128, :], atab[h, j + 1])

            def emit_av(r, tag='av'):
                j, h = r // HPC, r % HPC
                p_av = psp.tile([128, 512], F32, tag=tag,
                                bufs=2 if tag == 's3' else 1,
                                name=f'p_av_{r}')
                lst = ets.pop(r)
                for kc in range(KC):
                    nc.tensor.matmul(p_av[:], v4[:, kc, h, :],
                                     lst[kc // 3][:, kc % 3, :],
                                     start=(kc == 0), stop=(kc == KC - 1))
                    tick()
                rec = wkp.tile([64, 512], F32, tag='rec', name=f'rec_{r}')
                nc.vector.reciprocal(rec[:], p_av[64:128, :])
                nc.vector.tensor_mul(
                    t_ot[h // 2][(h % 2) * 64:(h % 2) * 64 + 64,
                                 j * 512:(j + 1) * 512],
                    p_av[0:64, :], rec[:])
                if h == HPC - 1:  # whole j-column normalized -> out-projection
                    late.extend(
                        [(lambda qq=qq, j=j:
                          emit_outproj(j, qq, tail=(j == QC - 1)), 5.0)
                         for qq in range(4)])

            # PE warm-up: ramp the clock out of pstate-low while the first
            # DMAs land; dummy matmuls on a memset tile, result never read
            # prefix: q/k m0-j0 so quad (j0,h0) can start; q borrows the idle
            # 'av' bank so k's matmuls don't wait on q's pack copies
            emit_q(0, 0, t_wq, t_bq if with_bias else None, t_qp, 'q',
                   tag='av')
            emit_q(0, 0, t_wk, t_bk if with_bias else None, t_kp, 'k')
            emit_v(0, tag='s3')
            emit_v(1, tag='s3')
            emit_rest_dmas()
            AVS = {4: (1,), 5: (2, 3), 10: (8, 9), 11: (10,)}
            for r in range(NR):
                state['late_ok'] = r >= 8
                state['r'] = r
                force(r)
                emit_sc(r)
                force(r + 0.5)
                for a in AVS.get(r, (r - 3,) if r == 3 else
                                 (r - LAG,) if r >= 5 else ()):
                    emit_av(a)
            emit_av(NR - 1, tag='mm')
            fill_all()

    _split_waits(nc)
    return nc


_NC_CACHE = {}


def _get_nc(with_bias=False):
    if with_bias not in _NC_CACHE:
        _NC_CACHE[with_bias] = _build(with_bias)
    return _NC_CACHE[with_bias]


def _host_prep(h, observation_state, Wq, bq, Wk, bk, Wv, bv, Wo, bo,
               Woq, boq, Wok, bok, variable_bias, relative_time_bias,
               with_bias=False):
    f32 = np.float32
    h = np.asarray(h, f32)
    obs = np.asarray(observation_state, f32).reshape(B, N, 2)
    Kidx = np.arange(N)
    tK = Kidx // V                                 # time bin of each token
    sq = np.float32(np.sqrt(SCALE))
    so = np.float32(np.sqrt(OBS_SCALE))
    kvar = (Kidx[None, :] % V == np.arange(V)[:, None]).astype(f32)  # [32,N]
    bq16 = ((Kidx[None, :] // V) % 16 == np.arange(16)[:, None]).astype(f32)

    # host obs projections (K=2 matmuls), sqrt(obs_scale) + bias folded
    oq = obs @ (np.asarray(Woq, f32) * so) + np.asarray(boq, f32) * so
    ok = obs @ (np.asarray(Wok, f32) * so) + np.asarray(bok, f32) * so

    Wq_s = np.asarray(Wq, f32) * sq
    Wk_s = np.asarray(Wk, f32) * sq

    def dev_w(w):  # [512, F] -> [128, 4, F] device layout
        return np.ascontiguousarray(
            w.reshape(4, 128, w.shape[1]).transpose(1, 0, 2)).astype(NPBF)

    in_maps = []
    for c in range(NCORES):
        b, hg = divmod(c, 2)
        h0 = hg * HPC
        cs, ce = h0 * HD, (h0 + HPC) * HD
        qt = np.empty((HPC, 64, N), f32)
        kt = np.empty((HPC, 48, N), f32)
        at = np.empty((HPC, QC, 16, N), f32)
        for hh in range(HPC):
            head = h0 + hh
            vb = np.asarray(variable_bias[head], f32)
            rtb = np.asarray(relative_time_bias[head], f32)
            qt[hh, 0:16] = oq[b, :, head * OD:(head + 1) * OD].T
            qt[hh, 16:48] = vb[Kidx % V, :].T          # VB_h[Q%32, r]
            qt[hh, 48:64] = bq16
            kt[hh, 0:16] = ok[b, :, head * OD:(head + 1) * OD].T
            kt[hh, 16:48] = kvar
            for j in range(QC):
                # A_hj[s, K] = rtb[16j + s - K//32 + 47]
                idx = 16 * j + np.arange(16)[:, None] - tK[None, :] + (T - 1)
                at[hh, j] = rtb[idx]
        m = {
            'fhT': dev_w(np.ascontiguousarray(h[b].reshape(N, D).T)),
            'wq': dev_w(Wq_s[:, cs:ce]),
            'wk': dev_w(Wk_s[:, cs:ce]),
            'wv': dev_w(np.asarray(Wv, f32)[:, cs:ce]),
            'wo': np.ascontiguousarray(
                np.asarray(Wo, f32)[cs:ce, :].reshape(2, 128, D)
                .transpose(1, 0, 2)).astype(NPBF),
            'qtab': qt.astype(NPBF),
            'ktab': kt.astype(NPBF),
            'atab': at.astype(NPBF),
            'vones': np.ones((128, 64), NPBF),
        }
        if with_bias:
            m.update({
                'bqr': (np.asarray(bq, f32)[None, cs:ce] * sq).astype(NPBF),
                'bkr': (np.asarray(bk, f32)[None, cs:ce] * sq).astype(NPBF),
                'bvr': np.asarray(bv, f32)[None, cs:ce].astype(NPBF),
                'onesd': np.ones((1, 512), NPBF),
            })
        in_maps.append(m)
    return in_maps


def kernel(**inputs):
    with_bias = any(
        np.any(np.asarray(inputs[k])) for k in ('bq', 'bk', 'bv'))
    nc = _get_nc(with_bias)
    in_maps = _host_prep(**inputs, with_bias=with_bias)
    res = run_bass_kernel_spmd(nc, in_maps, core_ids=list(range(NCORES)))
    bo = np.asarray(inputs['bo'], np.float32)
    outf = np.zeros((B, N, D), np.float32)
    for c in range(NCORES):
        outf[c // 2] += np.asarray(res.results[c]['out'], np.float32)
    outf += bo[None, None, :]
    return outf.reshape(B, T, V, D)


# revision 34
# speedup vs baseline: 1.4495x; 1.0219x over previous
"""Trainium2 Bass kernel for ClinicalStateFormationOperator.

Full-input contract: kernel(**inputs) takes the complete (unsharded) numpy
inputs and returns the full [B, T, V, D] output. Internally the work is
sharded across 8 NeuronCores as (batch, head-group): core c handles batch
c//2 and heads (c%2)*4 .. (c%2)*4+3. Each core computes its 4 heads'
attention and the partial output projection; the host sums the two partial
projections per batch and adds the output bias.

v6 design (v1 baseline 143.9us -> 99.3us cost-model time; rel err 8.8e-3):
 - Engine rebalance: Activation runs ONLY the 48 softmax exps (its cost-model
   floor, ~73us); psum->sbuf copies live on DVE; obs-state projections (K=2
   matmuls) are host prep; Pool/gpsimd cannot touch PSUM so it idles.
 - All operand tiles are bf16 (same PE rate as float32r in the cost model,
   half the DMA/SBUF): packs, E=exp(scores), v, attention-out, weights.
   Measured end-to-end rel err ~7.7e-3 vs the 2e-2 gate.
 - Software pipeline: round r emits the score matmuls + exps of quad r and
   (per the AVS table) the AV matmuls of a quad 2-3 rounds back. Stage-1
   projection / out-projection tasks drip from a deadline-guarded queue;
   consecutive fillers alternate between the 'mm' and (while free,
   rounds < 3) 'av' psum banks so each filler's psum->pack DVE copy
   overlaps the next filler's matmuls instead of stalling PE on the
   bank's write-after-read.
 - PSUM: 2x[128,3,512] score groups (6 banks) + 1 AV accumulator + 1
   proj/outproj bank = 8. Consecutive quads' AV accumulators ALTERNATE
   between the av and mm banks, so av(r+1) never waits for norm(r)'s DVE
   reciprocal+multiply to release its bank (this serial av->norm->av chain
   was the binding critical path at 101.5us; breaking it gave -2.2us).
   The prefix projections and the tail out-projections borrow the av/s3
   banks, which are idle at those times.
 - Rejected by measurement: fp8-DoubleRow scores (obs logits reach +-5.6;
   fp8's 3% rel err -> 24% output err) and fp8 E/v for AV (score row-max
   spans 0.44..10.1, no fixed exp-shift fits e4m3's window: best 3.1e-2
   vs the 2e-2 gate; a per-query shift is not expressible on ACT).
 - Weights/activations are DMA'd in device layout (host pre-transposed),
   first-needed first, split across the SP and ACT HWDGE queues.

Per-quad math (quad = (head h, 512-query chunk j), N = T*V = 1536 tokens):
scores are computed transposed (keys on partitions, queries free) in ONE
K=128 matmul per [128k x 512q] tile by packing four contraction groups into
the 128 pack rows:
    rows  0: 64  kT_h          |  qT_h            (content; sqrt(scale)
                                                   folded into Wq AND Wk)
    rows 64: 80  okT_h         |  oqT_h           (observation, host-computed
                                                   with sqrt(obs_scale) folded)
    rows 80:112  [K%32==r]     |  VB_h[Q%32, r]   (variable bias)
    rows112:128  A_hj[s,K]=rtb_h[16j+s-K//32+47] | [(Q//32)%16==s]  (time
                 bias; A rows re-DMA'd into the k-pack once per (h, j),
                 prefetched a full j-round ahead)
    E^T = exp(scores^T) in bf16  (|scores| <~ 6, fp32 psum in, no max-sub)
    [out^T; denom_rep] = [v_h | ones]^T @ E^T  (64 ones columns replicate
         the softmax denominator -> aligned DVE divide)
    OT = out^T * reciprocal(denom_rep)         (bf16)
    y_partial = OT^T_headpairs @ Wo_rows       (host sums core pairs + bo)
"""

from collections import deque

import numpy as np
import ml_dtypes

import concourse.bass as bass
import concourse.mybir as mybir
import concourse.tile as tile
from concourse.bass_utils import run_bass_kernel_spmd

V = 32
T = 48
D = 512
H = 8
HD = D // H          # 64
OD = 16
B = 4
N = T * V            # 1536
HPC = 4              # heads per core
NCORES = 8
SCALE = 1.0 / np.sqrt(HD)
OBS_SCALE = 1.0 / np.sqrt(OD)

F32 = mybir.dt.float32
BF16 = mybir.dt.bfloat16
NPBF = ml_dtypes.bfloat16
EXP = mybir.ActivationFunctionType.Exp

KC = N // 128        # 12 key chunks of 128
QC = N // 512        # 3 query chunks of 512
NR = HPC * QC        # 12 quads (rounds)
LAG = 2              # AV trails scores by 2 rounds


def _split_waits(nc, max_waits=1):
    """Walrus in this container allows only one sync-wait slot per
    instruction; spill extra waits onto preceding same-engine NoOps."""
    def fix_bb(bb):
        changed = False
        new = []
        for inst in bb.instructions:
            si = inst.sync_info
            if si is not None and len(si.on_wait) > max_waits:
                waits = list(si.on_wait)
                for w in waits[:-max_waits]:
                    new.append(mybir.InstNoOp(
                        name=nc.get_next_instruction_name(),
                        engine=inst.engine, ins=[], outs=[],
                        sync_info=mybir.SyncInfo(on_wait=[w], on_update=[])))
                    changed = True
                si.on_wait = waits[-max_waits:]
            new.append(inst)
        if changed:
            bb.instructions = new
        for sub in getattr(bb, 'blocks', []) or []:
            fix_bb(sub)
    for f in nc.m.functions:
        for bb in f.blocks:
            fix_bb(bb)


def _build(with_bias=False):
    nc = bass.Bass()

    # ---- per-core DRAM I/O, already in device layout (host transposes) ----
    fhT = nc.dram_tensor('fhT', [128, 4, N], BF16, kind='ExternalInput')
    wq = nc.dram_tensor('wq', [128, 4, HPC * HD], BF16, kind='ExternalInput')
    wk = nc.dram_tensor('wk', [128, 4, HPC * HD], BF16, kind='ExternalInput')
    wv = nc.dram_tensor('wv', [128, 4, HPC * HD], BF16, kind='ExternalInput')
    wo = nc.dram_tensor('wo', [128, 2, D], BF16, kind='ExternalInput')
    # static pack rows (host-built): qtab = [obs-q 16 | var-values 32 |
    # time-indicator 16] rows, ktab = [obs-k 16 | var-indicator 32]
    qtab = nc.dram_tensor('qtab', [HPC, 64, N], BF16, kind='ExternalInput')
    ktab = nc.dram_tensor('ktab', [HPC, 48, N], BF16, kind='ExternalInput')
    atab = nc.dram_tensor('atab', [HPC, QC, 16, N], BF16,
                          kind='ExternalInput')
    vones = nc.dram_tensor('vones', [128, 64], BF16, kind='ExternalInput')
    if with_bias:
        bqr = nc.dram_tensor('bqr', [1, HPC * HD], BF16, kind='ExternalInput')
        bkr = nc.dram_tensor('bkr', [1, HPC * HD], BF16, kind='ExternalInput')
        bvr = nc.dram_tensor('bvr', [1, HPC * HD], BF16, kind='ExternalInput')
        onesd = nc.dram_tensor('onesd', [1, 512], BF16, kind='ExternalInput')
    out = nc.dram_tensor('out', [N, D], BF16, kind='ExternalOutput')

    with tile.TileContext(nc) as tc:
        with tc.tile_pool(name='sb', bufs=1) as sb, \
             tc.tile_pool(name='etp', bufs=16) as etp, \
             tc.tile_pool(name='wkp', bufs=2) as wkp, \
             tc.tile_pool(name='psp', bufs=1, space='PSUM') as psp:

            t_fhT = sb.tile([128, 4, N], BF16)
            t_wq = sb.tile([128, 4, HPC * HD], BF16)
            t_wk = sb.tile([128, 4, HPC * HD], BF16)
            t_wv = sb.tile([128, 4, HPC * HD], BF16)
            t_wo = sb.tile([128, 2, D], BF16)
            t_qp = [sb.tile([128, N], BF16, name=f'qp{h}') for h in range(HPC)]
            t_kp = [sb.tile([128, N], BF16, name=f'kp{h}') for h in range(HPC)]
            # v packs: [keys, kc, head, 64 v-ch | 64 ones]
            v4 = sb.tile([128, KC, HPC, 128], BF16)
            t_ot = [sb.tile([128, N], BF16, name=f'ot{p}') for p in range(2)]
            if with_bias:
                t_bq = sb.tile([1, HPC * HD], BF16)
                t_bk = sb.tile([1, HPC * HD], BF16)
                t_bv = sb.tile([1, HPC * HD], BF16)
                t_ones = sb.tile([1, 512], BF16)

            # ---- critical DMAs only; the rest are emitted post-prefix ----
            nc.scalar.dma_start(t_wq[:], wq[:])
            for kk in range(4):
                eng = nc.sync if kk % 2 == 0 else nc.scalar
                eng.dma_start(t_fhT[:, kk, 0:512], fhT[:, kk, 0:512])
            nc.sync.dma_start(t_wk[:], wk[:])
            nc.sync.dma_start(t_kp[0][64:112, :], ktab[0])
            nc.sync.dma_start(t_qp[0][64:128, :], qtab[0])
            nc.sync.dma_start(t_kp[0][112:128, :], atab[0, 0])
            nc.scalar.dma_start(t_wv[:], wv[:])

            def emit_rest_dmas():
                # fhT j1/j2 feed the K(0,1)/K(0,2) fillers popped ~10-13us in
                for j in range(1, QC):
                    for kk in range(4):
                        eng = nc.sync if kk % 2 == 0 else nc.scalar
                        eng.dma_start(t_fhT[:, kk, j * 512:(j + 1) * 512],
                                      fhT[:, kk, j * 512:(j + 1) * 512])
                for h in range(1, HPC):
                    nc.sync.dma_start(t_kp[h][64:112, :], ktab[h])
                    nc.sync.dma_start(t_qp[h][64:128, :], qtab[h])
                    nc.sync.dma_start(t_kp[h][112:128, :], atab[h, 0])
                if with_bias:
                    nc.sync.dma_start(t_bq[:], bqr[:])
                    nc.sync.dma_start(t_bk[:], bkr[:])
                    nc.sync.dma_start(t_bv[:], bvr[:])
                    nc.sync.dma_start(t_ones[:], onesd[:])
                nc.sync.dma_start(t_wo[:], wo[:])
                for kc in range(KC):
                    nc.sync.dma_start(
                        v4[:, kc, :, 64:128],
                        vones[:, None, :].to_broadcast((128, HPC, 64)))

            # ---- stage-1 emitters (run as fillers inside the quad loop) ----
            def emit_q(m, j, w_t, b_t, packs, nm, tag='mm', on_act=False):
                p = psp.tile([128, 512], F32, tag=tag,
                             bufs=2 if tag == 's3' else 1, name=f'p_{nm}_{m}{j}')
                for kk in range(4):
                    nc.tensor.matmul(
                        p[:], w_t[:, kk, m * 128:(m + 1) * 128],
                        t_fhT[:, kk, j * 512:(j + 1) * 512],
                        start=(kk == 0), stop=(not with_bias and kk == 3))
                if with_bias:
                    nc.tensor.matmul(p[:], b_t[:, m * 128:(m + 1) * 128],
                                     t_ones[:], start=False, stop=True)
                for s in range(2):
                    dst = packs[2 * m + s][0:64, j * 512:(j + 1) * 512]
                    if on_act:  # ACT is idle during warm-up; unblock DVE
                        nc.scalar.copy(dst, p[s * 64:(s + 1) * 64, :])
                    else:
                        nc.vector.tensor_copy(dst, p[s * 64:(s + 1) * 64, :])

            def emit_v(kc, tag='mm'):
                p = psp.tile([128, HPC * HD], F32, tag=tag,
                             bufs=2 if tag == 's3' else 1, name=f'p_v{kc}')
                for kk in range(4):
                    nc.tensor.matmul(p[:], t_fhT[:, kk, kc * 128:(kc + 1) * 128],
                                     t_wv[:, kk, :], start=(kk == 0),
                                     stop=(not with_bias and kk == 3))
                if with_bias:
                    nc.tensor.matmul(p[:], t_ones[:, 0:128], t_bv[:],
                                     start=False, stop=True)
                nc.vector.tensor_copy(v4[:, kc, :, 0:64], p[:])

            def emit_outproj(j, qq, tail=False):
                qc = 4 * j + qq
                tag = ('s3', 's3', 'av', 'mm')[qq] if tail else 'mm'
                p = psp.tile([128, D], F32, tag=tag,
                             bufs=2 if tag == 's3' else 1, name=f'p_y{qc}')
                for pp in range(2):
                    nc.tensor.matmul(p[:], t_ot[pp][:, qc * 128:(qc + 1) * 128],
                                     t_wo[:, pp, :], start=(pp == 0),
                                     stop=(pp == 1))
                t_y = wkp.tile([128, D], BF16, tag='y', bufs=4, name=f't_y{qc}')
                if tail and qq % 2 == 0:
                    nc.scalar.copy(t_y[:], p[:])
                else:
                    nc.vector.tensor_copy(t_y[:], p[:])
                eng = nc.scalar if (tail and qq % 2 == 0) else nc.sync
                eng.dma_start(out[qc * 128:(qc + 1) * 128, :], t_y[:])

            fillers = deque()
            late = deque()          # out-projections, drained from round 8
            state = {'mm': 0, 'next': 3.0, 'popped': 0, 'late_ok': False,
                     'r': -1}

            def tick(k=1):
                state['mm'] += k
                while state['mm'] >= state['next']:
                    if fillers:
                        fn, sp, _ = fillers.popleft()
                    elif state['late_ok'] and late:
                        fn, sp = late.popleft()
                    else:
                        break
                    fn()
                    state['next'] += sp

            def force(dl):
                # hard deadline: emit every filler due before point `dl` NOW
                while fillers and fillers[0][2] <= dl:
                    fn, sp, _ = fillers.popleft()
                    fn()
                    state['next'] += sp

            def fill_all():
                while fillers:
                    fillers.popleft()[0]()
                while late:
                    late.popleft()[0]()

            def alt_tag(i):
                # 2nd bank is free until av(0) claims it in round 3
                return 'av' if (i % 2 and state['r'] < 3) else 'mm'

            def Q(m, j):
                return lambda: emit_q(m, j, t_wq, t_bq if with_bias else None,
                                      t_qp, 'q', tag=alt_tag(j + 1))

            def K(m, j, on_act=False):
                return lambda: emit_q(m, j, t_wk, t_bk if with_bias else None,
                                      t_kp, 'k', tag=alt_tag(j),
                                      on_act=on_act)

            def Vt(kc):
                return lambda: emit_v(kc, tag=alt_tag(kc))

            # deadline-ordered: K0* before quad(j0,h0) groups; Q10/K1* before
            # quad(j0,h2); all V before av(0) at round 2; Q*1/Q*2 before j1/j2
            fillers.extend(
                [(K(0, 1), 2.4, 0), (K(0, 2), 2.4, 0), (Q(1, 0), 2.4, 2),
                 (K(1, 0), 2.4, 2), (K(1, 1), 2.4, 2),
                 (K(1, 2), 2.4, 2)]
                + [(Vt(kc), 5.5, 3.5) for kc in range(2, KC)]
                + [(Q(0, 1), 2.4, 4), (Q(1, 1), 2.4, 6), (Q(0, 2), 2.4, 8),
                   (Q(1, 2), 2.4, 10)])

            # ---- software-pipelined quad rounds ----
            ets = {}

            def emit_sc(r):
                j, h = r // HPC, r % HPC
                lst = []
                for g in range(4):
                    p_s3 = psp.tile([128, 3, 512], F32, tag='s3', bufs=2,
                                    name=f'p_s3_{r}_{g}')
                    for i3 in range(3):
                        kc = 3 * g + i3
                        nc.tensor.matmul(
                            p_s3[:, i3, :],
                            t_kp[h][:, kc * 128:(kc + 1) * 128],
                            t_qp[h][:, j * 512:(j + 1) * 512],
                            start=True, stop=True)
                        tick()
                    et = etp.tile([128, 3, 512], BF16, tag='et',
                                  name=f'et_{r}_{g}')
                    nc.scalar.activation(et[:], p_s3[:], EXP)
                    lst.append(et)
                ets[r] = lst
                if j + 1 < QC:  # prefetch next j-round's time-bias rows
                    nc.sync.dma_start(t_kp[h][112:128, :], atab[h, j + 1])

            def emit_av(r, tag=None):
                # alternate the accumulator between the 'av' and 'mm' banks:
                # consecutive quads' AVs then never share a bank, so av(r+1)
                # does not wait for norm(r)'s DVE reciprocal+multiply reads
                if tag is None:
                    tag = 'av' if r % 2 == 0 else 'mm'
                j, h = r // HPC, r % HPC
                p_av = psp.tile([128, 512], F32, tag=tag,
                                bufs=2 if tag == 's3' else 1,
                                name=f'p_av_{r}')
                lst = ets.pop(r)
                for kc in range(KC):
                    nc.tensor.matmul(p_av[:], v4[:, kc, h, :],
                                     lst[kc // 3][:, kc % 3, :],
                                     start=(kc == 0), stop=(kc == KC - 1))
                    tick()
                rec = wkp.tile([64, 512], F32, tag='rec', name=f'rec_{r}')
                nc.vector.reciprocal(rec[:], p_av[64:128, :])
                nc.vector.tensor_mul(
                    t_ot[h // 2][(h % 2) * 64:(h % 2) * 64 + 64,
                                 j * 512:(j + 1) * 512],
                    p_av[0:64, :], rec[:])
                if h == HPC - 1:  # whole j-column normalized -> out-projection
                    late.extend(
                        [(lambda qq=qq, j=j:
                          emit_outproj(j, qq, tail=(j == QC - 1)), 5.0)
                         for qq in range(4)])

            # PE warm-up: ramp the clock out of pstate-low while the first
            # DMAs land; dummy matmuls on a memset tile, result never read
            # prefix: q/k m0-j0 so quad (j0,h0) can start; q borrows the idle
            # 'av' bank so k's matmuls don't wait on q's pack copies
            emit_q(0, 0, t_wq, t_bq if with_bias else None, t_qp, 'q',
                   tag='av')
            emit_q(0, 0, t_wk, t_bk if with_bias else None, t_kp, 'k')
            emit_v(0, tag='s3')
            emit_v(1, tag='s3')
            emit_rest_dmas()
            AVS = {4: (1,), 5: (2, 3), 10: (8, 9), 11: (10,)}
            for r in range(NR):
                state['late_ok'] = r >= 8
                state['r'] = r
                force(r)
                emit_sc(r)
                force(r + 0.5)
                for a in AVS.get(r, (r - 3,) if r == 3 else
                                 (r - LAG,) if r >= 5 else ()):
                    emit_av(a)
            emit_av(NR - 1, tag='mm')
            fill_all()

    _split_waits(nc)
    return nc


_NC_CACHE = {}


def _get_nc(with_bias=False):
    if with_bias not in _NC_CACHE:
        _NC_CACHE[with_bias] = _build(with_bias)
    return _NC_CACHE[with_bias]


def _host_prep(h, observation_state, Wq, bq, Wk, bk, Wv, bv, Wo, bo,
               Woq, boq, Wok, bok, variable_bias, relative_time_bias,
               with_bias=False):
    f32 = np.float32
    h = np.asarray(h, f32)
    obs = np.asarray(observation_state, f32).reshape(B, N, 2)
    Kidx = np.arange(N)
    tK = Kidx // V                                 # time bin of each token
    sq = np.float32(np.sqrt(SCALE))
    so = np.float32(np.sqrt(OBS_SCALE))
    kvar = (Kidx[None, :] % V == np.arange(V)[:, None]).astype(f32)  # [32,N]
    bq16 = ((Kidx[None, :] // V) % 16 == np.arange(16)[:, None]).astype(f32)

    # host obs projections (K=2 matmuls), sqrt(obs_scale) + bias folded
    oq = obs @ (np.asarray(Woq, f32) * so) + np.asarray(boq, f32) * so
    ok = obs @ (np.asarray(Wok, f32) * so) + np.asarray(bok, f32) * so

    Wq_s = np.asarray(Wq, f32) * sq
    Wk_s = np.asarray(Wk, f32) * sq

    def dev_w(w):  # [512, F] -> [128, 4, F] device layout
        return np.ascontiguousarray(
            w.reshape(4, 128, w.shape[1]).transpose(1, 0, 2)).astype(NPBF)

    in_maps = []
    for c in range(NCORES):
        b, hg = divmod(c, 2)
        h0 = hg * HPC
        cs, ce = h0 * HD, (h0 + HPC) * HD
        qt = np.empty((HPC, 64, N), f32)
        kt = np.empty((HPC, 48, N), f32)
        at = np.empty((HPC, QC, 16, N), f32)
        for hh in range(HPC):
            head = h0 + hh
            vb = np.asarray(variable_bias[head], f32)
            rtb = np.asarray(relative_time_bias[head], f32)
            qt[hh, 0:16] = oq[b, :, head * OD:(head + 1) * OD].T
            qt[hh, 16:48] = vb[Kidx % V, :].T          # VB_h[Q%32, r]
            qt[hh, 48:64] = bq16
            kt[hh, 0:16] = ok[b, :, head * OD:(head + 1) * OD].T
            kt[hh, 16:48] = kvar
            for j in range(QC):
                # A_hj[s, K] = rtb[16j + s - K//32 + 47]
                idx = 16 * j + np.arange(16)[:, None] - tK[None, :] + (T - 1)
                at[hh, j] = rtb[idx]
        m = {
            'fhT': dev_w(np.ascontiguousarray(h[b].reshape(N, D).T)),
            'wq': dev_w(Wq_s[:, cs:ce]),
            'wk': dev_w(Wk_s[:, cs:ce]),
            'wv': dev_w(np.asarray(Wv, f32)[:, cs:ce]),
            'wo': np.ascontiguousarray(
                np.asarray(Wo, f32)[cs:ce, :].reshape(2, 128, D)
                .transpose(1, 0, 2)).astype(NPBF),
            'qtab': qt.astype(NPBF),
            'ktab': kt.astype(NPBF),
            'atab': at.astype(NPBF),
            'vones': np.ones((128, 64), NPBF),
        }
        if with_bias:
            m.update({
                'bqr': (np.asarray(bq, f32)[None, cs:ce] * sq).astype(NPBF),
                'bkr': (np.asarray(bk, f32)[None, cs:ce] * sq).astype(NPBF),
                'bvr': np.asarray(bv, f32)[None, cs:ce].astype(NPBF),
                'onesd': np.ones((1, 512), NPBF),
            })
        in_maps.append(m)
    return in_maps


def kernel(**inputs):
    with_bias = any(
        np.any(np.asarray(inputs[k])) for k in ('bq', 'bk', 'bv'))
    nc = _get_nc(with_bias)
    in_maps = _host_prep(**inputs, with_bias=with_bias)
    res = run_bass_kernel_spmd(nc, in_maps, core_ids=list(range(NCORES)))
    bo = np.asarray(inputs['bo'], np.float32)
    outf = np.zeros((B, N, D), np.float32)
    for c in range(NCORES):
        outf[c // 2] += np.asarray(res.results[c]['out'], np.float32)
    outf += bo[None, None, :]
    return outf.reshape(B, T, V, D)
